# revision 24
# baseline (speedup 1.0000x reference)
"""Trainium2 Bass kernel for a 3-branch GCN layer (sum of three GCNConvs).

Math: out[b,t] = sum_k A_k @ (x[b,t] @ W_k) + b_k = x[b,t] @ Mop + bias where
Mop = sum_k kron(A_k^T, W_k) is [1600 x 1600] and block-sparse: block
(n_in, n_out) = sum_k A_k[n_out, n_in] W_k is nonzero only where some graph
has edge n_in->n_out (~30% of the 625 blocks, self-loops included).

Device strategy (data-parallel over batch across 8 cores):
- Host pre-transposes x into feature-major 128-row tiles so the PE does NO
  transposes: per row tile a [128, 13*128] fp16 slab whose j-th 128-column
  slice is the stationary lhsT for feature-pair chunk j. Input nodes are
  paired two-per-chunk by a max-weight matching on output-support overlap,
  minimizing the number of nonzero pair-blocks streamed.
- Mop is packed on host: per chunk only the nonzero 128x64 blocks, in n_out
  order, concatenated into one [128, MT] fp16 buffer.
- Per row tile: one matmul per run of consecutive nonzero n_out blocks,
  accumulating into 4 psum banks (n_out groups of 8). Zero blocks are never
  streamed. DVE adds bias during psum->SBUF drain with fp16 output.
- Mop chunks are interleaved across both HWDGE rings ahead of the x tiles,
  dummy warm-up matmuls run during the DMA lead-in to lift the PE HAM
  throttle, and the last tile drains bank-major to shorten the tail.
"""

import sys

import numpy as np

if "/opt/trn_rl_repo" not in sys.path:
    sys.path.insert(0, "/opt/trn_rl_repo")

B, T, NN, C = 64, 300, 25, 64
F = NN * C            # 1600
N_CORES = 8
BT_LOC = (B // N_CORES) * T   # 2400
NTILES = 19
PBT = NTILES * 128    # 2432 (rows padded with zeros)
NJ = 13               # feature chunks (12 node pairs + 1 singleton)
NBANK = 4             # psum banks: n_out groups [0:8),[8:16),[16:24),[24]
GROUP = 8             # n_out blocks per psum bank (8*64 = 512 fp32 = 1 bank)

_PROGRAM_CACHE = {}
_RUN_KW = {}


def _dense_adj(edge_index_k: np.ndarray) -> np.ndarray:
    """PyG GCNConv normalized dense adjacency A[dst, src] (float64)."""
    row = edge_index_k[0].astype(np.int64)
    col = edge_index_k[1].astype(np.int64)
    loop = np.arange(NN, dtype=np.int64)
    row = np.concatenate([row, loop])
    col = np.concatenate([col, loop])
    deg = np.zeros(NN, dtype=np.float64)
    np.add.at(deg, col, 1.0)
    dinv = np.where(deg > 0, 1.0 / np.sqrt(deg), 0.0)
    norm = dinv[row] * dinv[col]
    A = np.zeros((NN, NN), dtype=np.float64)
    np.add.at(A, (col, row), norm)
    return A


def _pair_nodes(supp):
    """Max-weight matching on |S_a & S_b| (minimizes total pair-blocks)."""
    try:
        import networkx as nx

        G = nx.Graph()
        for a in range(NN):
            for b in range(a + 1, NN):
                G.add_edge(a, b, weight=len(supp[a] & supp[b]))
        m = nx.max_weight_matching(G, maxcardinality=True)
        pairs = [tuple(sorted(p)) for p in m]
        matched = {n for p in pairs for n in p}
        single = (set(range(NN)) - matched).pop()
    except Exception:
        # greedy fallback
        rem = set(range(NN))
        pairs = []
        while len(rem) > 1:
            bp, bov = None, -1
            for a in sorted(rem):
                for b in sorted(rem):
                    if a < b and len(supp[a] & supp[b]) > bov:
                        bov, bp = len(supp[a] & supp[b]), (a, b)
            pairs.append(bp)
            rem -= set(bp)
        single = rem.pop()
    # big chunks first so tile-0 consumption matches DMA arrival order
    pairs.sort(key=lambda p: -len(supp[p[0]] | supp[p[1]]))
    return pairs, single


def _make_plan(union: np.ndarray):
    supp = [frozenset(np.nonzero(union[:, i])[0]) for i in range(NN)]
    pairs, single = _pair_nodes(supp)
    # singleton chunk first: it is the smallest mop transfer, so the first
    # matmul's DMA gate is minimal; the rest stay big-first so tile-0
    # consumption tracks arrival order.
    chunks = [(single, None, sorted(supp[single]))]
    chunks += [(a, b, sorted(supp[a] | supp[b])) for a, b in pairs]

    runs = []
    qoff = 0
    for (_a, _b, blocks) in chunks:
        rj = []
        i = 0
        while i < len(blocks):
            j = i + 1
            while (
                j < len(blocks)
                and blocks[j] == blocks[j - 1] + 1
                and blocks[j] // GROUP == blocks[i] // GROUP
            ):
                j += 1
            rj.append((blocks[i] // GROUP, blocks[i], j - i, qoff))
            qoff += (j - i) * C
            i = j
        runs.append(rj)
    return chunks, runs, qoff


def _build_operator(edge_index, W1, W2, W3, b1, b2, b3):
    """Host-side numeric prep: packed Mop blocks, bias row, layout plan."""
    Ws = [np.asarray(W, dtype=np.float64) for W in (W1, W2, W3)]
    bs = [np.asarray(b, dtype=np.float64) for b in (b1, b2, b3)]
    As = [_dense_adj(np.asarray(edge_index)[k]) for k in range(3)]
    union = (As[0] != 0) | (As[1] != 0) | (As[2] != 0)  # [n_out, n_in]

    chunks, runs, MT = _make_plan(union)

    mopc = np.zeros((128, MT), dtype=np.float64)

    def blk(n_in, n_out):
        out = np.zeros((C, C), dtype=np.float64)
        for k in range(3):
            if As[k][n_out, n_in] != 0:
                out += As[k][n_out, n_in] * Ws[k]
        return out

    for j, (a, b, _blocks) in enumerate(chunks):
        for (_bank, blk0, nblk, qoff) in runs[j]:
            for q in range(nblk):
                n_out = blk0 + q
                mopc[0:C, qoff + q * C : qoff + (q + 1) * C] = blk(a, n_out)
                if b is not None:
                    mopc[C:128, qoff + q * C : qoff + (q + 1) * C] = blk(b, n_out)

    bias = np.zeros(C, dtype=np.float64)
    for k in range(3):
        bias += bs[k]
    biasvec = np.tile(bias, NN).astype(np.float32)[None, :]  # [1, F]

    perm = np.zeros((NJ, 128), dtype=np.int64)
    pmask = np.zeros((NJ, 128), dtype=bool)
    for j, (a, b, _blocks) in enumerate(chunks):
        perm[j, 0:C] = a * C + np.arange(C)
        pmask[j, 0:C] = True
        if b is not None:
            perm[j, C:128] = b * C + np.arange(C)
            pmask[j, C:128] = True
    return chunks, runs, MT, mopc.astype(np.float16), biasvec, perm, pmask


def _prep_x(x16: np.ndarray, perm, pmask):
    """x16: [BT_LOC*N_CORES, F] fp16 -> per-core [PBT, NJ*128] slab layout:
    row t*128+p, col j*128+r  =  x[core, t*128+r, perm[j, p]]."""
    xs = x16.reshape(N_CORES, BT_LOC, F)
    g = xs[:, :, perm]                      # [N_CORES, BT_LOC, NJ, 128]
    g = g * pmask[None, None, :, :]
    gpad = np.zeros((N_CORES, PBT, NJ, 128), dtype=np.float16)
    gpad[:, :BT_LOC] = g
    out = gpad.reshape(N_CORES, NTILES, 128, NJ, 128).transpose(0, 1, 4, 3, 2)
    return np.ascontiguousarray(out).reshape(N_CORES, PBT, NJ * 128)


def _build_program(runs, MT):
    import concourse.bass as bass  # noqa: F401
    import concourse.tile as tile
    from concourse import bacc, mybir

    f32 = mybir.dt.float32
    f16 = mybir.dt.float16

    nc = bacc.Bacc(
        "TRN2", target_bir_lowering=False, debug=False, num_devices=N_CORES
    )
    xt = nc.dram_tensor("xt", [PBT, NJ * 128], f16, kind="ExternalInput").ap()
    mopc = nc.dram_tensor("mopc", [128, MT], f16, kind="ExternalInput").ap()
    biasvec = nc.dram_tensor("biasvec", [1, F], f32, kind="ExternalInput").ap()
    out = nc.dram_tensor("out", [PBT, F], f16, kind="ExternalOutput").ap()

    first_touch = {}
    last_touch = {}
    for s in range(NBANK):
        seq = [
            (j, ri)
            for j in range(NJ)
            for ri, r in enumerate(runs[j])
            if r[0] == s
        ]
        assert seq, f"psum bank {s} never written"
        first_touch[seq[0]] = True
        last_touch[seq[-1]] = True

    # last tile: bank-major emission, so recompute flags in that order
    lt_first = {}
    lt_last = {}
    for s in range(NBANK):
        seq = [
            (j, ri)
            for j in range(NJ)
            for ri, r in enumerate(runs[j])
            if r[0] == s
        ]
        lt_first[seq[0]] = True
        lt_last[seq[-1]] = True

    with tile.TileContext(nc) as tc:
        with (
            tc.tile_pool(name="const", bufs=1) as const_pool,
            tc.tile_pool(name="xin", bufs=4) as xin_pool,
            tc.tile_pool(name="outp", bufs=3) as out_pool,
            tc.tile_pool(name="po", bufs=2, space="PSUM") as po_pool,
        ):
            # ---- HAM warm-up: dummy matmuls on a garbage tile keep the PE
            # busy through the DMA lead-in so real matmuls start at 2.4 GHz.
            # contiguous ~3.8us of warm-up matmuls: one full HAM SHORT window
            # of PE-busy, so the clock gate lifts to 2.4 GHz right as the
            # first data-dependent matmuls become ready.
            wsrc = const_pool.tile([128, 512], f16, tag="wsrc", name="wsrc")
            nc.vector.memset(wsrc[:], 1.0)
            wps = po_pool.tile([128, 512], f32, tag="po0", name="wps")
            for _ in range(6):
                nc.tensor.matmul(
                    wps[:, :512], wsrc[:, :128], wsrc[:, :512],
                    start=True, stop=True,
                )

            # ---- constants + first x tile, interleaved across both rings
            def mop_w(j):
                return sum(r[2] for r in runs[j]) * C

            mop_sb = [None] * NJ
            for j in range(NJ):
                mop_sb[j] = (
                    const_pool.tile(
                        [128, mop_w(j)], f16, tag=f"mop{j}", name=f"mop{j}"
                    ),
                    runs[j][0][3],
                )

            xparts = [(0, 4), (4, 8), (8, NJ)]

            def xpieces(t):
                return [
                    (
                        const_pool.tile(
                            [128, (j1 - j0) * 128], f16, tag=f"x{t}_{i}",
                            name=f"x{t}_{i}",
                        ),
                        j0,
                        j1,
                    )
                    for i, (j0, j1) in enumerate(xparts)
                ]

            x0t = xpieces(0)

            def dma_mop(eng, j):
                t, q0 = mop_sb[j]
                eng.dma_start(t[:], mopc[:, q0 : q0 + mop_w(j)])

            def dma_xp(eng, pieces, t, i):
                tl, j0, j1 = pieces[i]
                eng.dma_start(
                    tl[:], xt[t * 128 : (t + 1) * 128, j0 * 128 : j1 * 128]
                )

            # ring split ~44/56 by measured HWDGE speeds (sync 185, scalar
            # 234 GB/s) so both finish the preload together; x1 rides the
            # sync ring right after its mop share so tile 1 is never gated.
            bias1 = const_pool.tile([1, F], f32, tag="bias1", name="bias1")
            bias_sb = const_pool.tile([128, F], f32, tag="bias", name="bias_sb")

            dma_xp(nc.sync, x0t, 0, 0)
            dma_mop(nc.sync, 0)
            dma_mop(nc.scalar, 1)
            dma_xp(nc.scalar, x0t, 0, 1)
            dma_mop(nc.sync, 2)
            dma_mop(nc.scalar, 3)
            dma_xp(nc.scalar, x0t, 0, 2)
            dma_mop(nc.sync, 5)
            dma_mop(nc.scalar, 4)
            dma_mop(nc.scalar, 6)
            dma_mop(nc.sync, 9)
            dma_mop(nc.scalar, 7)
            dma_mop(nc.sync, 11)
            dma_mop(nc.scalar, 8)
            nc.scalar.dma_start(bias1[:], biasvec[:])
            dma_mop(nc.scalar, 10)
            dma_mop(nc.scalar, 12)
            nc.gpsimd.partition_broadcast(bias_sb[:], bias1[:])

            def open_pos():
                return [
                    po_pool.tile([128, 512], f32, tag=f"po{s}", name=f"po{s}")
                    for s in range(NBANK)
                ]

            def emit_mm_for(pos, slabs, j, ri, first, last):
                bank, blk0, nblk, qoff = runs[j][ri]
                for (slab, j0, j1) in slabs:
                    if j0 <= j < j1:
                        lhsT = slab[:, (j - j0) * 128 : (j - j0 + 1) * 128]
                        break
                mt, q0 = mop_sb[j]
                c0 = (blk0 - bank * GROUP) * C
                nc.tensor.matmul(
                    pos[bank][:, c0 : c0 + nblk * C],
                    lhsT,
                    mt[:, qoff - q0 : qoff - q0 + nblk * C],
                    start=first,
                    stop=last,
                )

            def drain(pos, outt, t, eng):
                for s in range(NBANK):
                    w = min(512, F - s * 512)
                    nc.vector.tensor_add(
                        outt[:, s * 512 : s * 512 + w],
                        pos[s][:, :w],
                        bias_sb[:, s * 512 : s * 512 + w],
                    )
                eng.dma_start(out[t * 128 : (t + 1) * 128, :], outt[:])

            def emit_tile(t, slabs, bank_major):
                pos = open_pos()

                def emit_mm(j, ri, first, last):
                    emit_mm_for(pos, slabs, j, ri, first, last)

                outt = out_pool.tile([128, F], f16, tag="o", name="o")
                if not bank_major:
                    for j in range(NJ):
                        for ri in range(len(runs[j])):
                            emit_mm(
                                j,
                                ri,
                                first_touch.get((j, ri), False),
                                last_touch.get((j, ri), False),
                            )
                    drain(pos, outt, t, nc.scalar)
                else:
                    # bank-major: drain + DMA each bank as soon as it stops
                    for s in range(NBANK):
                        for j in range(NJ):
                            for ri in range(len(runs[j])):
                                if runs[j][ri][0] != s:
                                    continue
                                emit_mm(
                                    j,
                                    ri,
                                    lt_first.get((j, ri), False),
                                    lt_last.get((j, ri), False),
                                )
                        w = min(512, F - s * 512)
                        nc.vector.tensor_add(
                            outt[:, s * 512 : s * 512 + w],
                            pos[s][:, :w],
                            bias_sb[:, s * 512 : s * 512 + w],
                        )
                        eng = nc.sync if s % 2 else nc.scalar
                        eng.dma_start(
                            out[t * 128 : (t + 1) * 128, s * 512 : s * 512 + w],
                            outt[:, s * 512 : s * 512 + w],
                        )

            emit_tile(0, x0t, False)
            for t in range(1, NTILES):
                xt_t = xin_pool.tile([128, NJ * 128], f16, tag="x", name="x")
                nc.sync.dma_start(xt_t[:], xt[t * 128 : (t + 1) * 128, :])
                emit_tile(t, [(xt_t, 0, NJ)], t == NTILES - 1)

    nc.compile()
    return nc


def kernel(x, edge_index, W1, W2, W3, b1, b2, b3):
    from concourse.bass_utils import run_bass_kernel_spmd

    x16 = np.asarray(x, dtype=np.float32).astype(np.float16).reshape(-1, F)
    edge_index = np.asarray(edge_index)

    key = edge_index.tobytes()
    if _PROGRAM_CACHE.get("key") != key:
        chunks, runs, MT, mopc, biasvec, perm, pmask = _build_operator(
            edge_index, W1, W2, W3, b1, b2, b3
        )
        _PROGRAM_CACHE.update(
            key=key,
            nc=_build_program(runs, MT),
            plan=(chunks, runs, MT, mopc, biasvec, perm, pmask),
        )
    chunks, runs, MT, mopc, biasvec, perm, pmask = _PROGRAM_CACHE["plan"]
    nc = _PROGRAM_CACHE["nc"]

    xts = _prep_x(x16, perm, pmask)
    in_maps = [
        {"xt": xts[i], "mopc": mopc, "biasvec": biasvec}
        for i in range(N_CORES)
    ]
    res = run_bass_kernel_spmd(nc, in_maps, list(range(N_CORES)), **_RUN_KW)
    _PROGRAM_CACHE["last_result"] = res
    out = np.stack(
        [res.results[i]["out"][:BT_LOC] for i in range(N_CORES)], axis=0
    )
    return np.ascontiguousarray(
        out.reshape(B, T, NN, C).astype(np.float32)
    )


# revision 25
# speedup vs baseline: 1.0551x; 1.0551x over previous
"""Trainium2 Bass kernel for a 3-branch GCN layer (sum of three GCNConvs).

Math: out[b,t] = sum_k A_k @ (x[b,t] @ W_k) + b_k = x[b,t] @ Mop + bias where
Mop = sum_k kron(A_k^T, W_k) is [1600 x 1600] and block-sparse: block
(n_in, n_out) = sum_k A_k[n_out, n_in] W_k is nonzero only where some graph
has edge n_in->n_out (~30% of the 625 blocks, self-loops included).

Device strategy (data-parallel over batch across 8 cores):
- Host pre-transposes x into feature-major 128-row tiles so the PE does NO
  transposes: per row tile a [128, 13*128] fp16 slab whose j-th 128-column
  slice is the stationary lhsT for feature-pair chunk j. Input nodes are
  paired two-per-chunk by a max-weight matching on output-support overlap,
  minimizing the number of nonzero pair-blocks streamed.
- Mop is packed on host: per chunk only the nonzero 128x64 blocks, in n_out
  order, concatenated into one [128, MT] fp16 buffer.
- Per row tile: one matmul per run of consecutive nonzero n_out blocks,
  accumulating into 4 psum banks (n_out groups of 8). Zero blocks are never
  streamed. DVE adds bias during psum->SBUF drain with fp16 output.
- Mop chunks are interleaved across both HWDGE rings ahead of the x tiles,
  dummy warm-up matmuls run during the DMA lead-in to lift the PE HAM
  throttle, and the last tile drains bank-major to shorten the tail.
"""

import sys

import numpy as np

if "/opt/trn_rl_repo" not in sys.path:
    sys.path.insert(0, "/opt/trn_rl_repo")

B, T, NN, C = 64, 300, 25, 64
F = NN * C            # 1600
N_CORES = 8
BT_LOC = (B // N_CORES) * T   # 2400
NTILES = 19
PBT = NTILES * 128    # 2432 (rows padded with zeros)
NJ = 13               # feature chunks (12 node pairs + 1 singleton)
NBANK = 4             # psum banks: n_out groups [0:8),[8:16),[16:24),[24]
GROUP = 8             # n_out blocks per psum bank (8*64 = 512 fp32 = 1 bank)

_PROGRAM_CACHE = {}
_RUN_KW = {}


def _dense_adj(edge_index_k: np.ndarray) -> np.ndarray:
    """PyG GCNConv normalized dense adjacency A[dst, src] (float64)."""
    row = edge_index_k[0].astype(np.int64)
    col = edge_index_k[1].astype(np.int64)
    loop = np.arange(NN, dtype=np.int64)
    row = np.concatenate([row, loop])
    col = np.concatenate([col, loop])
    deg = np.zeros(NN, dtype=np.float64)
    np.add.at(deg, col, 1.0)
    dinv = np.where(deg > 0, 1.0 / np.sqrt(deg), 0.0)
    norm = dinv[row] * dinv[col]
    A = np.zeros((NN, NN), dtype=np.float64)
    np.add.at(A, (col, row), norm)
    return A


def _pair_nodes(supp):
    """Max-weight matching on |S_a & S_b| (minimizes total pair-blocks)."""
    try:
        import networkx as nx

        G = nx.Graph()
        for a in range(NN):
            for b in range(a + 1, NN):
                G.add_edge(a, b, weight=len(supp[a] & supp[b]))
        m = nx.max_weight_matching(G, maxcardinality=True)
        pairs = [tuple(sorted(p)) for p in m]
        matched = {n for p in pairs for n in p}
        single = (set(range(NN)) - matched).pop()
    except Exception:
        # greedy fallback
        rem = set(range(NN))
        pairs = []
        while len(rem) > 1:
            bp, bov = None, -1
            for a in sorted(rem):
                for b in sorted(rem):
                    if a < b and len(supp[a] & supp[b]) > bov:
                        bov, bp = len(supp[a] & supp[b]), (a, b)
            pairs.append(bp)
            rem -= set(bp)
        single = rem.pop()
    # big chunks first so tile-0 consumption matches DMA arrival order
    pairs.sort(key=lambda p: -len(supp[p[0]] | supp[p[1]]))
    return pairs, single


def _make_plan(union: np.ndarray):
    supp = [frozenset(np.nonzero(union[:, i])[0]) for i in range(NN)]
    pairs, single = _pair_nodes(supp)
    # singleton chunk first: it is the smallest mop transfer, so the first
    # matmul's DMA gate is minimal; the rest stay big-first so tile-0
    # consumption tracks arrival order.
    chunks = [(single, None, sorted(supp[single]))]
    chunks += [(a, b, sorted(supp[a] | supp[b])) for a, b in pairs]

    runs = []
    qoff = 0
    for (_a, _b, blocks) in chunks:
        rj = []
        i = 0
        while i < len(blocks):
            j = i + 1
            while (
                j < len(blocks)
                and blocks[j] == blocks[j - 1] + 1
                and blocks[j] // GROUP == blocks[i] // GROUP
            ):
                j += 1
            rj.append((blocks[i] // GROUP, blocks[i], j - i, qoff))
            qoff += (j - i) * C
            i = j
        runs.append(rj)
    return chunks, runs, qoff


def _build_operator(edge_index, W1, W2, W3, b1, b2, b3):
    """Host-side numeric prep: packed Mop blocks, bias row, layout plan."""
    Ws = [np.asarray(W, dtype=np.float64) for W in (W1, W2, W3)]
    bs = [np.asarray(b, dtype=np.float64) for b in (b1, b2, b3)]
    As = [_dense_adj(np.asarray(edge_index)[k]) for k in range(3)]
    union = (As[0] != 0) | (As[1] != 0) | (As[2] != 0)  # [n_out, n_in]

    chunks, runs, MT = _make_plan(union)

    mopc = np.zeros((128, MT), dtype=np.float64)

    def blk(n_in, n_out):
        out = np.zeros((C, C), dtype=np.float64)
        for k in range(3):
            if As[k][n_out, n_in] != 0:
                out += As[k][n_out, n_in] * Ws[k]
        return out

    for j, (a, b, _blocks) in enumerate(chunks):
        for (_bank, blk0, nblk, qoff) in runs[j]:
            for q in range(nblk):
                n_out = blk0 + q
                mopc[0:C, qoff + q * C : qoff + (q + 1) * C] = blk(a, n_out)
                if b is not None:
                    mopc[C:128, qoff + q * C : qoff + (q + 1) * C] = blk(b, n_out)

    bias = np.zeros(C, dtype=np.float64)
    for k in range(3):
        bias += bs[k]
    biasvec = np.tile(bias, NN).astype(np.float32)[None, :]  # [1, F]

    perm = np.zeros((NJ, 128), dtype=np.int64)
    pmask = np.zeros((NJ, 128), dtype=bool)
    for j, (a, b, _blocks) in enumerate(chunks):
        perm[j, 0:C] = a * C + np.arange(C)
        pmask[j, 0:C] = True
        if b is not None:
            perm[j, C:128] = b * C + np.arange(C)
            pmask[j, C:128] = True
    return chunks, runs, MT, mopc.astype(np.float16), biasvec, perm, pmask


def _prep_x(x16: np.ndarray, perm, pmask):
    """x16: [BT_LOC*N_CORES, F] fp16 -> per-core [PBT, NJ*128] slab layout:
    row t*128+p, col j*128+r  =  x[core, t*128+r, perm[j, p]]."""
    xs = x16.reshape(N_CORES, BT_LOC, F)
    g = xs[:, :, perm]                      # [N_CORES, BT_LOC, NJ, 128]
    g = g * pmask[None, None, :, :]
    gpad = np.zeros((N_CORES, PBT, NJ, 128), dtype=np.float16)
    gpad[:, :BT_LOC] = g
    out = gpad.reshape(N_CORES, NTILES, 128, NJ, 128).transpose(0, 1, 4, 3, 2)
    return np.ascontiguousarray(out).reshape(N_CORES, PBT, NJ * 128)


def _build_program(runs, MT):
    import concourse.bass as bass  # noqa: F401
    import concourse.tile as tile
    from concourse import bacc, mybir

    f32 = mybir.dt.float32
    f16 = mybir.dt.float16

    nc = bacc.Bacc(
        "TRN2", target_bir_lowering=False, debug=False, num_devices=N_CORES
    )
    xt = nc.dram_tensor("xt", [PBT, NJ * 128], f16, kind="ExternalInput").ap()
    mopc = nc.dram_tensor("mopc", [128, MT], f16, kind="ExternalInput").ap()
    biasvec = nc.dram_tensor("biasvec", [1, F], f32, kind="ExternalInput").ap()
    out = nc.dram_tensor("out", [PBT, F], f16, kind="ExternalOutput").ap()

    first_touch = {}
    last_touch = {}
    for s in range(NBANK):
        seq = [
            (j, ri)
            for j in range(NJ)
            for ri, r in enumerate(runs[j])
            if r[0] == s
        ]
        assert seq, f"psum bank {s} never written"
        first_touch[seq[0]] = True
        last_touch[seq[-1]] = True

    # last tile: bank-major emission, so recompute flags in that order
    lt_first = {}
    lt_last = {}
    for s in range(NBANK):
        seq = [
            (j, ri)
            for j in range(NJ)
            for ri, r in enumerate(runs[j])
            if r[0] == s
        ]
        lt_first[seq[0]] = True
        lt_last[seq[-1]] = True

    with tile.TileContext(nc) as tc:
        with (
            tc.tile_pool(name="const", bufs=1) as const_pool,
            tc.tile_pool(name="xin", bufs=4) as xin_pool,
            tc.tile_pool(name="outp", bufs=3) as out_pool,
            tc.tile_pool(name="po", bufs=2, space="PSUM") as po_pool,
        ):
            # ---- HAM warm-up: dummy matmuls on a garbage tile keep the PE
            # busy through the DMA lead-in so real matmuls start at 2.4 GHz.
            # contiguous ~3.8us of warm-up matmuls: one full HAM SHORT window
            # of PE-busy, so the clock gate lifts to 2.4 GHz right as the
            # first data-dependent matmuls become ready.
            wsrc = const_pool.tile([128, 512], f16, tag="wsrc", name="wsrc")
            nc.vector.memset(wsrc[:], 1.0)
            wps = po_pool.tile([128, 512], f32, tag="po0", name="wps")
            for _ in range(6):
                nc.tensor.matmul(
                    wps[:, :512], wsrc[:, :128], wsrc[:, :512],
                    start=True, stop=True,
                )

            # ---- constants + first x tile, interleaved across both rings
            def mop_w(j):
                return sum(r[2] for r in runs[j]) * C

            mop_sb = [None] * NJ
            for j in range(NJ):
                mop_sb[j] = (
                    const_pool.tile(
                        [128, mop_w(j)], f16, tag=f"mop{j}", name=f"mop{j}"
                    ),
                    runs[j][0][3],
                )

            xparts = [(0, 4), (4, 8), (8, NJ)]

            def xpieces(t):
                return [
                    (
                        const_pool.tile(
                            [128, (j1 - j0) * 128], f16, tag=f"x{t}_{i}",
                            name=f"x{t}_{i}",
                        ),
                        j0,
                        j1,
                    )
                    for i, (j0, j1) in enumerate(xparts)
                ]

            x0t = xpieces(0)

            def dma_mop(eng, j):
                t, q0 = mop_sb[j]
                eng.dma_start(t[:], mopc[:, q0 : q0 + mop_w(j)])

            def dma_xp(eng, pieces, t, i):
                tl, j0, j1 = pieces[i]
                eng.dma_start(
                    tl[:], xt[t * 128 : (t + 1) * 128, j0 * 128 : j1 * 128]
                )

            dma_xp(nc.sync, x0t, 0, 0)
            dma_mop(nc.sync, 0)
            dma_mop(nc.scalar, 1)
            dma_mop(nc.sync, 2)
            dma_xp(nc.scalar, x0t, 0, 1)
            dma_mop(nc.sync, 3)
            dma_mop(nc.scalar, 4)
            dma_mop(nc.sync, 5)
            dma_xp(nc.sync, x0t, 0, 2)
            dma_mop(nc.scalar, 6)
            dma_mop(nc.sync, 7)
            dma_mop(nc.scalar, 8)
            dma_mop(nc.sync, 9)
            dma_mop(nc.scalar, 10)
            dma_mop(nc.sync, 11)
            dma_mop(nc.scalar, 12)

            bias1 = const_pool.tile([1, F], f32, tag="bias1", name="bias1")
            nc.scalar.dma_start(bias1[:], biasvec[:])
            bias_sb = const_pool.tile([128, F], f32, tag="bias", name="bias_sb")
            nc.gpsimd.partition_broadcast(bias_sb[:], bias1[:])

            def open_pos():
                return [
                    po_pool.tile([128, 512], f32, tag=f"po{s}", name=f"po{s}")
                    for s in range(NBANK)
                ]

            def emit_mm_for(pos, slabs, j, ri, first, last):
                bank, blk0, nblk, qoff = runs[j][ri]
                for (slab, j0, j1) in slabs:
                    if j0 <= j < j1:
                        lhsT = slab[:, (j - j0) * 128 : (j - j0 + 1) * 128]
                        break
                mt, q0 = mop_sb[j]
                c0 = (blk0 - bank * GROUP) * C
                nc.tensor.matmul(
                    pos[bank][:, c0 : c0 + nblk * C],
                    lhsT,
                    mt[:, qoff - q0 : qoff - q0 + nblk * C],
                    start=first,
                    stop=last,
                )

            def drain(pos, outt, t, eng):
                for s in range(NBANK):
                    w = min(512, F - s * 512)
                    nc.vector.tensor_add(
                        outt[:, s * 512 : s * 512 + w],
                        pos[s][:, :w],
                        bias_sb[:, s * 512 : s * 512 + w],
                    )
                eng.dma_start(out[t * 128 : (t + 1) * 128, :], outt[:])

            def emit_tile(t, slabs, bank_major):
                pos = open_pos()

                def emit_mm(j, ri, first, last):
                    emit_mm_for(pos, slabs, j, ri, first, last)

                outt = out_pool.tile([128, F], f16, tag="o", name="o")
                if not bank_major:
                    for j in range(NJ):
                        for ri in range(len(runs[j])):
                            emit_mm(
                                j,
                                ri,
                                first_touch.get((j, ri), False),
                                last_touch.get((j, ri), False),
                            )
                    drain(pos, outt, t, nc.scalar)
                else:
                    # bank-major: drain + DMA each bank as soon as it stops
                    for s in range(NBANK):
                        for j in range(NJ):
                            for ri in range(len(runs[j])):
                                if runs[j][ri][0] != s:
                                    continue
                                emit_mm(
                                    j,
                                    ri,
                                    lt_first.get((j, ri), False),
                                    lt_last.get((j, ri), False),
                                )
                        w = min(512, F - s * 512)
                        nc.vector.tensor_add(
                            outt[:, s * 512 : s * 512 + w],
                            pos[s][:, :w],
                            bias_sb[:, s * 512 : s * 512 + w],
                        )
                        eng = nc.sync if s % 2 else nc.scalar
                        eng.dma_start(
                            out[t * 128 : (t + 1) * 128, s * 512 : s * 512 + w],
                            outt[:, s * 512 : s * 512 + w],
                        )

            emit_tile(0, x0t, False)
            for t in range(1, NTILES):
                xt_t = xin_pool.tile([128, NJ * 128], f16, tag="x", name="x")
                nc.sync.dma_start(xt_t[:], xt[t * 128 : (t + 1) * 128, :])
                emit_tile(t, [(xt_t, 0, NJ)], t == NTILES - 1)

    nc.compile()
    return nc


def kernel(x, edge_index, W1, W2, W3, b1, b2, b3):
    from concourse.bass_utils import run_bass_kernel_spmd

    x16 = np.asarray(x, dtype=np.float32).astype(np.float16).reshape(-1, F)
    edge_index = np.asarray(edge_index)

    key = edge_index.tobytes()
    if _PROGRAM_CACHE.get("key") != key:
        chunks, runs, MT, mopc, biasvec, perm, pmask = _build_operator(
            edge_index, W1, W2, W3, b1, b2, b3
        )
        _PROGRAM_CACHE.update(
            key=key,
            nc=_build_program(runs, MT),
            plan=(chunks, runs, MT, mopc, biasvec, perm, pmask),
        )
    chunks, runs, MT, mopc, biasvec, perm, pmask = _PROGRAM_CACHE["plan"]
    nc = _PROGRAM_CACHE["nc"]

    xts = _prep_x(x16, perm, pmask)
    in_maps = [
        {"xt": xts[i], "mopc": mopc, "biasvec": biasvec}
        for i in range(N_CORES)
    ]
    res = run_bass_kernel_spmd(nc, in_maps, list(range(N_CORES)), **_RUN_KW)
    _PROGRAM_CACHE["last_result"] = res
    out = np.stack(
        [res.results[i]["out"][:BT_LOC] for i in range(N_CORES)], axis=0
    )
    return np.ascontiguousarray(
        out.reshape(B, T, NN, C).astype(np.float32)
    )
